# revision 6
# baseline (speedup 1.0000x reference)
"""AdEx neuron Euler integration on 8 TRN2 NeuronCores.

Strategy (pure data parallel over neurons, 128 per core = SBUF partitions):
the 40000-step time recurrence is solved per time-block by Picard
iteration: given a guessed V trajectory for the block, the exp term and
spike masks are evaluated elementwise (parallel in time), which makes the
V/w recurrences affine time-varying; those are solved EXACTLY by the
hardware `tensor_tensor_scan` (state = d0*state + d1, 1 elem/cycle).
Iterating K times per block converges to the exact sequential fp32
trajectory (spike resets make the dynamics strongly self-correcting).

Recurrence (pre-state emitted at each step k):
  no spike: V' = alpha*V + E + gamma*w + c_k,  E = exp(sV + bE0)
  spike (V > th): V' = Vres
  w' = p*w + q*V + r + b*spike   ->  shifted: wh = w - Winf (kills r)
"""

import os
import sys

for _p in ("/opt/trn_rl_repo", "/opt/pypackages"):
    if _p not in sys.path:
        sys.path.insert(0, _p)

import numpy as np

import concourse.bass as bass
import concourse.bacc as bacc
import concourse.mybir as mybir
import concourse.tile as tile
from concourse.bass_utils import run_bass_kernel_spmd
from concourse import dve_ops as _dve_ops
from concourse.dve_ops import DveOp, OPS, _SUB_OPCODE_FOR_NAME, _CUSTOM_DVE_ROW_BASE
from concourse.dve_spec import Spec, Src0, Src1, C0, C1, C2, select, lower, _has_src1
from concourse.dve_uop import DveOpSpec

f32 = np.float32
T_STEPS = 40000
N_NEURONS = 1024
NCORES = 8
P = 128

LAST_EXEC_NS = None  # set when ADEX_TRACE=1
LAST_RESULTS = None


def _register_op(name, spec):
    """Register a custom DVE op at runtime (sha computed by lowering)."""
    if name in _SUB_OPCODE_FOR_NAME:
        for op in OPS:
            if op.name == name:
                return op
        raise RuntimeError(name)
    opcode = _CUSTOM_DVE_ROW_BASE + len(OPS)
    shas = {}
    for ver in ("v3", "v4"):
        shas[ver] = DveOpSpec(
            name=name, opcode=opcode, uops=lower(spec, ver=ver),
            rd1_en=_has_src1(spec),
        ).sha(ver)
    op = DveOp(name, spec, subdim=False, uops_sha=shas)
    OPS.append(op)
    _SUB_OPCODE_FOR_NAME[name] = opcode
    return op


# d1 = select(E > Ethr, Vres, E + zzc)       [in0=E, in1=zzc, s0=Ethr, s1=Vres]
ADEX_D1 = _register_op(
    "ADEX_D1",
    Spec(
        body=select(Src0 > C0, C1, Src0 + Src1),
        reference=lambda in0, in1, s0, s1, imm2: np.where(
            in0 > s0, s1, in0 + in1
        ).astype(np.float32),
    ),
)

# u = q*V + b*(E > Ethr)                      [in0=E, in1=V, s0=Ethr, s1=b, imm2=q]
ADEX_U = _register_op(
    "ADEX_U",
    Spec(
        body=Src1 * C2 + C1 * (Src0 > C0),
        reference=lambda in0, in1, s0, s1, imm2: (
            in1 * imm2 + s1 * (in0 > s0)
        ).astype(np.float32),
    ),
)


def _block_plan(c_all, dt, tau):
    """Split [0,T) into blocks; c (the per-step drive constant) must be
    uniform within each block. Returns [(k0, blen, c_blk, n_sweeps)]."""
    chg = (np.nonzero(np.diff(c_all))[0] + 1).tolist()
    bounds = [0] + chg + [len(c_all)]
    # drive strong enough that the equilibrium sits in the runaway region
    c_hot = f32(dt / tau) * f32(-0.048)
    plan = []
    for si in range(len(bounds) - 1):
        s0, s1 = bounds[si], bounds[si + 1]
        c_blk = f32(c_all[s0])
        hot = c_blk > c_hot
        seg_len = s1 - s0
        B = 500 if hot else 750
        nb = max(1, (seg_len + B - 1) // B)
        sizes = [seg_len // nb + (1 if i < seg_len % nb else 0) for i in range(nb)]
        k0 = s0
        for bi, blen in enumerate(sizes):
            if hot:
                K = 16
            else:
                # spikes can carry over into the first blocks of a quiet segment
                K = 16 if (si > 0 and bi < 2) else 5
            plan.append((k0, blen, float(c_blk), K))
            k0 += blen
    assert k0 == len(c_all)
    return plan


def _build(plan, consts, w_every=2):
    """Build the per-core Bass graph (SPMD: same program all cores)."""
    AF = mybir.ActivationFunctionType
    ALU = mybir.AluOpType
    T = sum(b for (_, b, _, _) in plan)
    Bmax = max(b for (_, b, _, _) in plan)

    nc = bacc.Bacc("TRN2", target_bir_lowering=False, debug=False,
                   num_devices=NCORES)
    v0_d = nc.dram_tensor("v0", [P, 1], mybir.dt.float32, kind="ExternalInput").ap()
    w0_d = nc.dram_tensor("w0h", [P, 1], mybir.dt.float32, kind="ExternalInput").ap()
    vout = nc.dram_tensor("vout", [P, T], mybir.dt.float32, kind="ExternalOutput").ap()
    wout = nc.dram_tensor("wout", [P, T], mybir.dt.float32, kind="ExternalOutput").ap()

    cs = consts
    with tile.TileContext(nc) as tc:
        with tc.tile_pool(name="persist", bufs=1) as pool:
            VB = pool.tile([P, Bmax + 1], mybir.dt.float32)   # V_{k0..k0+B}
            WS = pool.tile([P, Bmax + 1], mybir.dt.float32)   # wh_{k0..k0+B}
            VC = pool.tile([P, 1], mybir.dt.float32)          # V carry
            WC = pool.tile([P, 1], mybir.dt.float32)          # wh carry
            E = pool.tile([P, Bmax], mybir.dt.float32)
            d0 = pool.tile([P, Bmax], mybir.dt.float32)
            d1 = pool.tile([P, Bmax], mybir.dt.float32)
            u = pool.tile([P, Bmax], mybir.dt.float32)
            zzc = pool.tile([P, Bmax], mybir.dt.float32)
            wE = pool.tile([P, Bmax], mybir.dt.float32)
            pT = pool.tile([P, Bmax], mybir.dt.float32)       # constant p

            bET = pool.tile([P, 1], mybir.dt.float32)
            nc.vector.memset(bET[:], cs["bE0"])
            nc.vector.memset(pT[:], cs["p"])
            nc.sync.dma_start(VC[:], v0_d[:])
            nc.sync.dma_start(WC[:], w0_d[:])
            nc.scalar.copy(VB[:, 0:1], VC[:])
            nc.scalar.copy(WS[:, 0:1], WC[:])

            first = True
            for (k0, B, c_blk, K) in plan:
                # flat initial guess: broadcast the carry along the block
                nc.scalar.activation(
                    VB[:, 1:B], pT[:, 0 : B - 1], AF.Identity,
                    bias=VC[:], scale=0.0,
                )
                zzc_c = float(f32(f32(c_blk) + f32(cs["gamma"]) * f32(cs["Winf"])))
                for s in range(K):
                    nc.scalar.activation(
                        E[:, :B], VB[:, 0:B], AF.Exp,
                        bias=bET[:], scale=cs["s_exp"],
                    )
                    if s % w_every == 0:
                        # wh-scan input u = q*V + b*(E > Ethr)
                        nc.vector._custom_dve(
                            ADEX_U, out=u[:, :B], in0=E[:, :B], in1=VB[:, 0:B],
                            s0=cs["Ethr"], s1=cs["b"], imm2=cs["q"],
                        )
                        nc.vector.tensor_tensor_scan(
                            WS[:, 1 : B + 1], pT[:, :B], u[:, :B], WC[:],
                            ALU.mult, ALU.add,
                        )
                        # zzc = gamma*wh_pre + (c + gamma*Winf)
                        nc.vector.tensor_scalar(
                            zzc[:, :B], WS[:, 0:B], cs["gamma"], zzc_c,
                            ALU.mult, ALU.add,
                        )
                    # d0 = (E <= Ethr) * alpha
                    nc.vector.tensor_scalar(
                        d0[:, :B], E[:, :B], cs["Ethr"], cs["alpha"],
                        ALU.is_le, ALU.mult,
                    )
                    # d1 = select(E > Ethr, Vres, E + zzc)
                    nc.vector._custom_dve(
                        ADEX_D1, out=d1[:, :B], in0=E[:, :B], in1=zzc[:, :B],
                        s0=cs["Ethr"], s1=cs["Vres"],
                    )
                    nc.vector.tensor_tensor_scan(
                        VB[:, 1 : B + 1], d0[:, :B], d1[:, :B], VC[:],
                        ALU.mult, ALU.add,
                    )
                # final w pass with the converged V trajectory
                nc.scalar.activation(
                    E[:, :B], VB[:, 0:B], AF.Exp,
                    bias=bET[:], scale=cs["s_exp"],
                )
                nc.vector._custom_dve(
                    ADEX_U, out=u[:, :B], in0=E[:, :B], in1=VB[:, 0:B],
                    s0=cs["Ethr"], s1=cs["b"], imm2=cs["q"],
                )
                nc.vector.tensor_tensor_scan(
                    WS[:, 1 : B + 1], pT[:, :B], u[:, :B], WC[:],
                    ALU.mult, ALU.add,
                )
                # emitted w = wh + Winf
                nc.vector.tensor_scalar(
                    wE[:, :B], WS[:, 0:B], cs["Winf"], None, ALU.add,
                )
                nc.sync.dma_start(vout[:, k0 : k0 + B], VB[:, 0:B])
                nc.sync.dma_start(wout[:, k0 : k0 + B], wE[:, :B])
                # carries
                nc.scalar.copy(VC[:], VB[:, B : B + 1])
                nc.scalar.copy(WC[:], WS[:, B : B + 1])
                nc.scalar.copy(VB[:, 0:1], VC[:])
                nc.scalar.copy(WS[:, 0:1], WC[:])
                first = False
    nc.compile()
    return nc


def _derive_consts(V_rest, V_reset, V_T, V_thres, delta_T, R, tau, tau_w, a, b):
    dt = f32(5e-5)
    alpha = f32(1) - dt / f32(tau)
    beta = dt * f32(delta_T) / f32(tau)
    gamma = -(dt * f32(R) / f32(tau))
    p = f32(1) - dt / f32(tau_w)
    q = dt * f32(a) / f32(tau_w)
    r = -q * f32(V_rest)
    s_exp = f32(1.0) / f32(delta_T)
    bE0 = f32(np.log(beta) - f32(V_T) / f32(delta_T))
    Ethr = f32(np.exp(s_exp * f32(V_thres) + bE0))
    Winf = f32(r / (dt / f32(tau_w)))
    return dict(
        dt=float(dt), alpha=float(alpha), gamma=float(gamma), p=float(p),
        q=float(q), s_exp=float(s_exp), bE0=float(bE0), Ethr=float(Ethr),
        Winf=float(Winf), b=float(f32(b)), Vres=float(f32(V_reset)),
        tau=float(f32(tau)),
    )


def kernel(I_ext, V0, w0, V_rest, V_reset, V_T, V_thres, delta_T, R, tau,
           tau_w, a, b):
    global LAST_EXEC_NS, LAST_RESULTS
    I_ext = np.asarray(I_ext, f32)
    V0 = np.asarray(V0, f32)
    w0 = np.asarray(w0, f32)
    cs = _derive_consts(V_rest, V_reset, V_T, V_thres, delta_T, R, tau,
                        tau_w, a, b)
    dt = f32(cs["dt"])
    c_all = (dt / f32(tau) * (f32(V_rest) + f32(R) * I_ext[:T_STEPS])).astype(f32)
    plan = _block_plan(c_all, cs["dt"], cs["tau"])

    nc = _build(plan, cs)

    in_maps = []
    for c in range(NCORES):
        sl = slice(c * P, (c + 1) * P)
        in_maps.append({
            "v0": V0[sl].reshape(P, 1).copy(),
            "w0h": (w0[sl] - f32(cs["Winf"])).reshape(P, 1).copy(),
        })
    trace = os.environ.get("ADEX_TRACE", "0") == "1"
    res = run_bass_kernel_spmd(nc, in_maps, core_ids=list(range(NCORES)),
                               trace=trace)
    LAST_EXEC_NS = res.exec_time_ns
    LAST_RESULTS = res

    Vs = np.empty((T_STEPS, N_NEURONS), f32)
    ws = np.empty((T_STEPS, N_NEURONS), f32)
    for c in range(NCORES):
        sl = slice(c * P, (c + 1) * P)
        Vs[:, sl] = res.results[c]["vout"].T
        ws[:, sl] = res.results[c]["wout"].T
    return Vs, ws
